# revision 1
# baseline (speedup 1.0000x reference)
"""Causal multi-head self-attention on 8 Trainium2 NeuronCores.

Problem: x[4, 2048, 2048] fp32, w_q/w_k/w_v/w_o [2048, 2048] fp32,
16 heads x d_head=128. out = softmax(causal(QK^T/sqrt(128))) V @ w_o.

Sharding: tensor-parallel over heads. Core c owns heads {2c, 2c+1}:
computes Q^T/K^T ([d_head, tokens]) and V ([tokens, d_head]) for its
heads from the full x (streamed pre-transposed as x^T), runs per-head
causal attention producing attnT [d_head, tokens], then the partial
output o_c = attn_c @ w_o[head rows]; the host sums the 8 partials.

All matmuls run as float32r (full PE rate at moving-dim >= 256,
~1.4e-4 scale-relative precision). Softmax skips max-subtraction
(scores are O(+-6); exp is safe in fp32): probsT = exp(scale*scoresT)
is computed directly in [k, q] orientation (K^T stationary), so no
transposes are needed anywhere. Causal masking is a gpsimd
affine_select on the diagonal tiles. Row sums come from a ones-vector
matmul over DVE-accumulated probsT; 1/sum is broadcast across
partitions with a rank-1 PE matmul. The output projection is fused
into the attention loop (per 512-token q-superblock) to keep the PE
busy while the scalar engine works through the exps.
"""

import contextlib

import numpy as np

import concourse.bass as bass
from concourse import bacc
import concourse.mybir as mybir
from concourse.tile import TileContext
from concourse.bass_utils import run_bass_kernel_spmd

B = 4
S = 2048
D = 2048
NH = 16
DH = 128
N_CORES = 8
HPC = NH // N_CORES          # heads per core = 2
HD = HPC * DH                # head dims per core = 256
KO = D // 128                # contraction chunks = 16
TSB = 256                    # projection token superblock
NSB = S // TSB               # 8
QSB = 512                    # attention q superblock
NQSB = S // QSB              # 4
NKC = S // 128               # 16 k-chunks per batch
SCALE = float(1.0 / np.sqrt(DH))

F32 = mybir.dt.float32
F32R = mybir.dt.float32r

_CACHED = {}

# tunables for A/B experiments
NARROW = True
PSAT_BUFS = 1
PSO_TAG = "psO"
PSO_BUFS = 1
BCAST_GPSIMD = False
PHASES = "full"
PSSUM_TAG = "psSum"


def build(loop_r: int | None = None):
    nc = bacc.Bacc("TRN2", target_bir_lowering=False, debug=False)
    xT = nc.dram_tensor("xT", [B, D, S], F32, kind="ExternalInput")
    wq = nc.dram_tensor("wq", [D, HD], F32, kind="ExternalInput")
    wk = nc.dram_tensor("wk", [D, HD], F32, kind="ExternalInput")
    wv = nc.dram_tensor("wv", [D, HD], F32, kind="ExternalInput")
    wo = nc.dram_tensor("wo", [HD, D], F32, kind="ExternalInput")
    out = nc.dram_tensor("out", [B, S, D], F32, kind="ExternalOutput")

    env = {
        "xT_v": xT.ap().bitcast(F32R),
        "out": out,
    }

    with TileContext(nc) as tc:
        with tc.tile_pool(name="const", bufs=1) as const, \
             tc.tile_pool(name="big", bufs=1) as big, \
             tc.tile_pool(name="vpool", bufs=2) as vpool, \
             tc.tile_pool(name="stream", bufs=2) as stream, \
             tc.tile_pool(name="work", bufs=4) as work, \
             tc.tile_pool(name="ps", bufs=2, space="PSUM") as ps:

            # ---- weights / constants (resident) ----
            wq_sb = const.tile([128, KO, HD], F32R)
            wk_sb = const.tile([128, KO, HD], F32R)
            wv_sb = const.tile([128, KO, HD], F32R)
            wo_sb = const.tile([128, HPC, D], F32R)
            for (wt, wsb) in ((wq, wq_sb), (wk, wk_sb), (wv, wv_sb)):
                wv_ = wt.ap().rearrange("(ko p) m -> p ko m", p=128).bitcast(F32R)
                for ko in range(KO):
                    nc.sync.dma_start(wsb[:, ko], wv_[:, ko])
            nc.sync.dma_start(
                wo_sb, wo.ap().rearrange("(c p) n -> p c n", p=128).bitcast(F32R))
            ones_col = const.tile([128, 1], F32R)
            ones_row = const.tile([1, 128], F32R)
            tmp1 = const.tile([128, 1], F32)
            nc.vector.memset(tmp1, 1.0)
            nc.vector.tensor_copy(ones_col, tmp1)
            nc.vector.tensor_copy(ones_row, tmp1[0:1, 0:1].to_broadcast([1, 128]))

            env.update(wq_sb=wq_sb, wk_sb=wk_sb, wv_sb=wv_sb, wo_sb=wo_sb,
                       ones_col=ones_col, ones_row=ones_row,
                       big=big, vpool=vpool, stream=stream, work=work, ps=ps)

            loop_cm = (tc.For_i(0, loop_r, 1) if loop_r else
                       contextlib.nullcontext())
            with loop_cm:
                _batches(nc, env)

    nc.compile()
    return nc


def _batches(nc, env):
    wq_sb, wk_sb, wv_sb, wo_sb = (env["wq_sb"], env["wk_sb"], env["wv_sb"],
                                  env["wo_sb"])
    ones_col, ones_row = env["ones_col"], env["ones_row"]
    xT_v, out = env["xT_v"], env["out"]
    big, vpool, stream, work, ps = (env["big"], env["vpool"], env["stream"],
                                    env["work"], env["ps"])

    # qt is written/read at matching subtile granularity across batches, so
    # one resident tile overlaps batches via subtile WAR deps. kt/v regions
    # are read until the very end of a batch's attention, so they rotate
    # through 2 slots instead.
    qt_sb = big.tile([128, HPC, S], F32R, tag="qt", bufs=1)
    for b in range(B):
        # ===== projections =====
        kt_sb = vpool.tile([128, HPC, S], F32R, tag="kt", bufs=2)
        v_sb = vpool.tile([128, NKC, HD], F32R, tag="v", bufs=2)
        for s in range(NSB):
            xt = stream.tile([128, KO, TSB], F32R, tag="xt", bufs=2)
            for ko in range(KO):
                nc.sync.dma_start(
                    xt[:, ko],
                    xT_v[b, ko * 128:(ko + 1) * 128, s * TSB:(s + 1) * TSB])
            # Q^T, K^T tiles: [dh, tokens] = W_chunk^T @ x^T
            for m in range(HPC):
                psq = ps.tile([128, TSB], F32, tag="psProj", bufs=2)
                for ko in range(KO):
                    nc.tensor.matmul(
                        psq, wq_sb[:, ko, m * 128:(m + 1) * 128], xt[:, ko],
                        start=(ko == 0), stop=(ko == KO - 1))
                nc.scalar.copy(qt_sb[:, m, s * TSB:(s + 1) * TSB], psq)
                psk = ps.tile([128, TSB], F32, tag="psProj", bufs=2)
                for ko in range(KO):
                    nc.tensor.matmul(
                        psk, wk_sb[:, ko, m * 128:(m + 1) * 128], xt[:, ko],
                        start=(ko == 0), stop=(ko == KO - 1))
                nc.scalar.copy(kt_sb[:, m, s * TSB:(s + 1) * TSB], psk)
            # V tiles: [tokens, dh] = x @ W_v (x^T chunks stationary)
            for t in range(TSB // 128):
                tc_idx = s * (TSB // 128) + t
                psv = ps.tile([128, HD], F32, tag="psProj", bufs=2)
                for ko in range(KO):
                    nc.tensor.matmul(
                        psv, xt[:, ko, t * 128:(t + 1) * 128], wv_sb[:, ko, :],
                        start=(ko == 0), stop=(ko == KO - 1))
                nc.scalar.copy(v_sb[:, tc_idx, :], psv)

        if PHASES == "proj":
            continue
        # ===== attention + fused output projection, per q-superblock =====
        for j in range(NQSB):
            nkc = 4 * (j + 1)
            at_tiles = []
            for h in range(HPC):
                acc = work.tile([128, QSB], F32R, tag="acc", bufs=2)
                ps_at = ps.tile([128, QSB], F32, tag="psAT", bufs=PSAT_BUFS)
                for c in range(nkc):
                    # causal narrowing: on diagonal chunks only q-cols
                    # >= c*128 matter; keep matmul width >= 256.
                    if NARROW and c >= 4 * j:
                        off = min(c * 128 - j * QSB, QSB - 256)
                    else:
                        off = 0
                    w = QSB - off
                    ps_s = ps.tile([128, QSB], F32, tag="psS", bufs=2)
                    nc.tensor.matmul(
                        ps_s[:, off:], kt_sb[:, h, c * 128:(c + 1) * 128],
                        qt_sb[:, h, j * QSB + off:(j + 1) * QSB],
                        start=True, stop=True)
                    pt = work.tile([128, QSB], F32R, tag="pt", bufs=3)
                    nc.scalar.activation(
                        pt[:, off:], ps_s[:, off:],
                        mybir.ActivationFunctionType.Exp, scale=SCALE)
                    if c >= 4 * j:
                        # causal: keep only q - k >= 0 on diagonal tiles
                        nc.gpsimd.affine_select(
                            out=pt[:, off:], in_=pt[:, off:],
                            compare_op=mybir.AluOpType.is_ge,
                            fill=0.0,
                            base=j * QSB + off - c * 128,
                            pattern=[[1, w]],
                            channel_multiplier=-1)
                    if c == 0:
                        nc.vector.tensor_copy(acc, pt)
                    else:
                        nc.vector.tensor_add(
                            acc[:, off:], acc[:, off:], pt[:, off:])
                    nc.tensor.matmul(
                        ps_at[:, off:], v_sb[:, c, h * 128:(h + 1) * 128],
                        pt[:, off:],
                        start=(c == 0), stop=(c == nkc - 1))
                # softmax denominator (sum over k = partitions + chunks)
                ps_sum = ps.tile([1, QSB], F32, tag=PSSUM_TAG, bufs=PSO_BUFS if PSSUM_TAG == PSO_TAG else 1)
                nc.tensor.matmul(ps_sum, ones_col, acc, start=True, stop=True)
                inv_r = work.tile([1, QSB], F32R, tag="invr", bufs=2)
                with nc.allow_low_precision(
                        reason="f32r denominator: 2^-14 rounding is fine"):
                    nc.vector.reciprocal(inv_r, ps_sum)
                inv_bc = work.tile([128, QSB], F32, tag="invbc", bufs=2)
                if BCAST_GPSIMD:
                    nc.gpsimd.partition_broadcast(
                        inv_bc, inv_r.bitcast(F32))
                else:
                    ps_bc = ps.tile([128, QSB], F32, tag="psBC", bufs=1)
                    nc.tensor.matmul(ps_bc, ones_row, inv_r,
                                     start=True, stop=True)
                    nc.vector.tensor_copy(inv_bc, ps_bc)
                at = work.tile([128, QSB], F32R, tag="at", bufs=3)
                nc.vector.tensor_mul(at, ps_at, inv_bc)
                at_tiles.append(at)
            if PHASES == "noO":
                continue
            # fused O-projection for this q-superblock's 512 tokens
            for t in range(QSB // 128):
                for n in range(D // 512):
                    ps_o = ps.tile([128, 512], F32, tag=PSO_TAG, bufs=PSO_BUFS)
                    for h in range(HPC):
                        nc.tensor.matmul(
                            ps_o, at_tiles[h][:, t * 128:(t + 1) * 128],
                            wo_sb[:, h, n * 512:(n + 1) * 512],
                            start=(h == 0), stop=(h == HPC - 1))
                    o_st = work.tile([128, 512], F32, tag="ost", bufs=2)
                    nc.vector.tensor_copy(o_st, ps_o)
                    nc.sync.dma_start(
                        out.ap()[b, j * QSB + t * 128:j * QSB + (t + 1) * 128,
                                 n * 512:(n + 1) * 512], o_st)


def kernel(x, w_q, w_k, w_v, w_o, _trace=False):
    x = np.ascontiguousarray(np.asarray(x, dtype=np.float32))
    xT = np.ascontiguousarray(x.transpose(0, 2, 1))
    in_maps = []
    for c in range(N_CORES):
        sl = slice(c * HD, (c + 1) * HD)
        in_maps.append({
            "xT": xT,
            "wq": np.ascontiguousarray(np.asarray(w_q, np.float32)[:, sl]),
            "wk": np.ascontiguousarray(np.asarray(w_k, np.float32)[:, sl]),
            "wv": np.ascontiguousarray(np.asarray(w_v, np.float32)[:, sl]),
            "wo": np.ascontiguousarray(np.asarray(w_o, np.float32)[sl, :]),
        })
    if "nc" not in _CACHED:
        _CACHED["nc"] = build()
    res = run_bass_kernel_spmd(
        _CACHED["nc"], in_maps, core_ids=list(range(N_CORES)),
        trace=_trace)
    if _trace:
        _CACHED["last_result"] = res
    acc = np.zeros((B, S, D), dtype=np.float64)
    for r in res.results:
        acc += r["out"]
    return acc.astype(np.float32)



# revision 18
# speedup vs baseline: 1.2057x; 1.2057x over previous
"""Causal multi-head self-attention on 8 Trainium2 NeuronCores.

Problem: x[4, 2048, 2048] fp32, w_q/w_k/w_v/w_o [2048, 2048] fp32,
16 heads x d_head=128. out = softmax(causal(QK^T/sqrt(128))) V @ w_o.

Sharding: tensor-parallel over heads. Core c owns heads {2c, 2c+1}:
computes Q^T/K^T ([d_head, tokens]) and V ([tokens, d_head]) for its
heads from the full x (streamed pre-transposed as x^T), runs per-head
causal attention producing attnT [d_head, tokens], then the partial
output o_c = attn_c @ w_o[head rows]; the host sums the 8 partials.

v2 design notes (PE-occupancy focused — TRN2 drops the PE clock from
2.4 to 1.2 GHz after any stall, so the whole schedule is built to
keep the tensor engine continuously fed):
- Probabilities ride in fp16: exp(scale*s - 8) fits comfortably
  (scores are O(+-6)); denominators/numerators are both scaled by
  e^-8 so the ratio is unchanged. fp16 halves DVE cost (2x/4x modes)
  and is full-rate on the PE.
- Scores for BOTH heads of a chunk land in one [128, 2, 512] PSUM
  pair-tile -> ONE exp activation per chunk covers both heads, so the
  Act engine has 2x latency slack vs the PE's score+PV work.
- Causal masking multiplies by constant 0/1 fp16 mask tiles on the
  DVE (fast mode) instead of gpsimd affine_select per tile.
- Softmax denominator: fp16 chunk accumulation (DVE) then one PE
  matmul pair (selector columns) gives both heads' row sums in a
  [2, 512] PSUM tile; reciprocal_approx_fast (single DVE op, ~18
  bits) replaces the 3.3us-per-call exact reciprocal; a rank-1 PE
  matmul broadcasts 1/sum across partitions; the at = ps_at * inv
  multiply drains attention PSUM.
- O-projection of block j is deferred until after block j+1's first
  two score pairs are emitted, so the PE chews fresh score work while
  the denominator chain (DVE) finishes - no per-block PE bubble.
- PSUM->SBUF output drains alternate DVE/gpsimd so neither engine
  rate-limits the O-projection matmul stream.
- PSUM budget exactly 8 banks: psS 2x[128,2,512] (4) + psAT 2x[128,
  512] (2) + psO 2x[128,512] (2); projections reuse the psS tag.
"""

import contextlib
from collections import deque

import numpy as np

import concourse.bass as bass
from concourse import bacc
import concourse.mybir as mybir
from concourse.tile import TileContext
from concourse.bass_utils import run_bass_kernel_spmd

B = 4
S = 2048
D = 2048
NH = 16
DH = 128
N_CORES = 8
HPC = NH // N_CORES          # heads per core = 2
HD = HPC * DH                # head dims per core = 256
KO = D // 128                # contraction chunks = 16
TSB = 512                    # projection token superblock
NSB = S // TSB               # 4
QSB = 512                    # attention q superblock
NQSB = S // QSB              # 4
SCALE = float(1.0 / np.sqrt(DH))
CSHIFT = -8.0                # exp(s*scale - 8): keeps fp16 probs in range

F32 = mybir.dt.float32
F32R = mybir.dt.float32r
F16 = mybir.dt.float16

_CACHED = {}


def build(loop_r: int | None = None):
    nc = bacc.Bacc("TRN2", target_bir_lowering=False, debug=False)
    xT = nc.dram_tensor("xT", [B, D, S], F32, kind="ExternalInput")
    wq = nc.dram_tensor("wq", [D, HD], F32, kind="ExternalInput")
    wk = nc.dram_tensor("wk", [D, HD], F32, kind="ExternalInput")
    wv = nc.dram_tensor("wv", [D, HD], F32, kind="ExternalInput")
    wo = nc.dram_tensor("wo", [HD, D], F32, kind="ExternalInput")
    out = nc.dram_tensor("out", [B, S, D], F32, kind="ExternalOutput")

    env = {
        "xT_v": xT.ap().bitcast(F32R),
        "out": out,
    }

    with TileContext(nc) as tc:
        with tc.tile_pool(name="const", bufs=1) as const, \
             tc.tile_pool(name="big", bufs=1) as big, \
             tc.tile_pool(name="stream", bufs=2) as stream, \
             tc.tile_pool(name="work", bufs=4) as work, \
             tc.tile_pool(name="ps", bufs=2, space="PSUM") as ps:

            # ---- weights / constants (resident) ----
            wq_sb = const.tile([128, KO, HD], F32R)
            wk_sb = const.tile([128, KO, HD], F32R)
            wv_sb = const.tile([128, KO, HD], F32R)
            wo_sb = const.tile([128, HPC, D], F32R)

            env.update(wq_sb=wq_sb, wk_sb=wk_sb, wv_sb=wv_sb, wo_sb=wo_sb,
                       big=big, stream=stream, work=work, ps=ps)

            # Batch-0 superblock-0 xt is DMA'd interleaved with wq so the
            # first Q matmul chain starts ~2MB in, not after all weights.
            xt0 = stream.tile([128, KO, TSB], F32R, tag="xt", bufs=2)
            wq_v = wq.ap().rearrange("(ko p) m -> p ko m", p=128).bitcast(F32R)
            wk_v = wk.ap().rearrange("(ko p) m -> p ko m", p=128).bitcast(F32R)
            wv_v = wv.ap().rearrange("(ko p) m -> p ko m", p=128).bitcast(F32R)
            xT_v = env["xT_v"]
            for ko in range(KO):
                nc.sync.dma_start(wq_sb[:, ko], wq_v[:, ko])
                nc.sync.dma_start(xt0[:, ko], xT_v[0, ko * 128:(ko + 1) * 128,
                                                  0:TSB])
            for ko in range(KO):
                nc.sync.dma_start(wk_sb[:, ko], wk_v[:, ko])
            for ko in range(KO):
                nc.sync.dma_start(wv_sb[:, ko], wv_v[:, ko])
            nc.sync.dma_start(
                wo_sb, wo.ap().rearrange("(c p) n -> p c n", p=128).bitcast(F32R))
            env["xt0"] = xt0

            # ones_row for the rank-1 partition broadcast of 1/denominator
            tmp1 = const.tile([128, 1], F32)
            ones_row = const.tile([1, 128], F32R)
            nc.vector.memset(tmp1, 1.0)
            nc.vector.tensor_copy(ones_row, tmp1[0:1, 0:1].to_broadcast([1, 128]))
            # per-partition bias column for exp(s*scale + CSHIFT)
            cbias = const.tile([128, 1], F32)
            nc.vector.memset(cbias, CSHIFT)
            env["cbias"] = cbias

            # fp16 ones column for the per-head row-sum matmul
            ones_col = const.tile([128, 1], F16)
            nc.vector.tensor_copy(ones_col, tmp1)

            # causal masks (0/1), both heads' halves identical. For a
            # diagonal chunk at d = c*128 - j*512, column g of the q-block
            # is kept iff g >= d + p. One mask tile per d in {0,..,384}.
            mtmp = const.tile([128, 512], F32)
            masks = []
            for d in range(0, 512, 128):
                mk = const.tile([128, 2, 512], F16, name=f"mask{d}")
                nc.vector.memset(mtmp, 1.0)
                nc.gpsimd.affine_select(
                    out=mtmp, in_=mtmp, compare_op=mybir.AluOpType.is_ge,
                    fill=0.0, base=-d, pattern=[[1, 512]],
                    channel_multiplier=-1)
                nc.vector.tensor_copy(mk[:, 0], mtmp)
                nc.vector.tensor_copy(mk[:, 1], mtmp)
                masks.append(mk)

            env.update(ones_row=ones_row, ones_col=ones_col, masks=masks)

            loop_cm = (tc.For_i(0, loop_r, 1) if loop_r else
                       contextlib.nullcontext())
            with loop_cm:
                _batches(nc, env)

    nc.compile()
    return nc


def _proj(nc, env, b):
    """Q^T/K^T/V projections for batch b into qt/kt/v SBUF tiles."""
    stream, work, ps = env["stream"], env["work"], env["ps"]
    wq_sb, wk_sb, wv_sb = env["wq_sb"], env["wk_sb"], env["wv_sb"]
    xT_v = env["xT_v"]
    qt_sb, kt_sb, v_sb = env["qt_sb"], env["kt_sb"], env["v_sb"]

    for s in range(NSB):
        if b == 0 and s == 0:
            xt = env["xt0"]          # pre-loaded interleaved with wq
        else:
            xt = stream.tile([128, KO, TSB], F32R, tag="xt", bufs=2)
            for ko in range(KO):
                nc.sync.dma_start(
                    xt[:, ko],
                    xT_v[b, ko * 128:(ko + 1) * 128, s * TSB:(s + 1) * TSB])
        # Q^T then K^T: [dh, tokens] = W_chunk^T @ x^T; both 128-rows of
        # head-dim go in one [128, 2, 512] psum pair-tile.
        for (wsb, dst) in ((wq_sb, qt_sb), (wk_sb, kt_sb)):
            psq = ps.tile([128, 2, TSB], F32, tag="psS", bufs=2)
            for m in range(HPC):
                for ko in range(KO):
                    nc.tensor.matmul(
                        psq[:, m], wsb[:, ko, m * 128:(m + 1) * 128], xt[:, ko],
                        start=(ko == 0), stop=(ko == KO - 1))
            nc.scalar.copy(dst[:, :, s * TSB:(s + 1) * TSB], psq)
        # V: [tokens, dh] = x @ W_v, two 128-token chunks per pair-tile,
        # downcast to fp16 on drain.
        for tp in range(TSB // 256):
            psv = ps.tile([128, 2, TSB], F32, tag="psS", bufs=2)
            for ti in range(2):
                t = tp * 2 + ti
                for ko in range(KO):
                    nc.tensor.matmul(
                        psv[:, ti, 0:HD],
                        xt[:, ko, t * 128:(t + 1) * 128], wv_sb[:, ko],
                        start=(ko == 0), stop=(ko == KO - 1))
            tc0 = s * (TSB // 128) + tp * 2
            nc.scalar.copy(v_sb[:, tc0:tc0 + 2, :], psv[:, :, 0:HD])


def _o_proj(nc, env, b, j, at_tiles):
    """Fused output projection for q-superblock j of batch b."""
    work, ps = env["work"], env["ps"]
    wo_sb, out = env["wo_sb"], env["out"]
    for t in range(QSB // 128):
        for n in range(D // 512):
            ps_o = ps.tile([128, 512], F32, tag="psO", bufs=2)
            for h in range(HPC):
                nc.tensor.matmul(
                    ps_o, at_tiles[h][:, t * 128:(t + 1) * 128],
                    wo_sb[:, h, n * 512:(n + 1) * 512],
                    start=(h == 0), stop=(h == HPC - 1))
            o_st = work.tile([128, 512], F32, tag="ost", bufs=3)
            if (t * 4 + n) % 2 == 0:
                nc.vector.tensor_copy(o_st, ps_o)
            else:
                nc.scalar.copy(o_st, ps_o)
            nc.sync.dma_start(
                out.ap()[b, j * QSB + t * 128:j * QSB + (t + 1) * 128,
                         n * 512:(n + 1) * 512], o_st)


def _attn(nc, env, b):
    """Causal attention + deferred fused O-projection for batch b."""
    work, ps = env["work"], env["ps"]
    qt_sb, kt_sb, v_sb = env["qt_sb"], env["kt_sb"], env["v_sb"]
    ones_row, ones_col = env["ones_row"], env["ones_col"]
    masks = env["masks"]

    at_prev = None
    for j in range(NQSB):
        nkc = 4 * (j + 1)

        def score_pair(c):
            # causal narrowing: on diagonal chunks only q-cols >= c*128
            # matter; keep matmul width >= 256 for f32r full rate.
            if c >= 4 * j:
                off = min(c * 128 - j * QSB, QSB - 256)
            else:
                off = 0
            pss = ps.tile([128, 2, QSB], F32, tag="psS", bufs=2)
            for h in range(HPC):
                nc.tensor.matmul(
                    pss[:, h, off:], kt_sb[:, h, c * 128:(c + 1) * 128],
                    qt_sb[:, h, j * QSB + off:(j + 1) * QSB],
                    start=True, stop=True)
            return pss, off

        # two score pairs in flight before anything else: the PE chews
        # these while the previous block's denominator chain drains.
        pend = deque()
        pend.append(score_pair(0))
        if nkc > 1:
            pend.append(score_pair(1))
        if at_prev is not None:
            _o_proj(nc, env, b, j - 1, at_prev)

        acc = work.tile([128, 2, QSB], F16, tag="acc", bufs=2)
        ps_at = [ps.tile([128, QSB], F32, tag="psAT", bufs=2, name=f"psat{h}")
                 for h in range(HPC)]
        for c in range(nkc):
            pss, off = pend.popleft()
            pt = work.tile([128, 2, QSB], F16, tag="pt", bufs=3)
            nc.scalar.activation(
                pt[:, :, off:], pss[:, :, off:],
                mybir.ActivationFunctionType.Exp, bias=env["cbias"],
                scale=SCALE)
            if c >= 4 * j:
                # causal: multiply by the 0/1 mask for this diagonal chunk
                msk = masks[(c * 128 - j * QSB) // 128]
                nc.vector.tensor_mul(
                    pt[:, :, off:], pt[:, :, off:], msk[:, :, off:])
            if c == 0:
                nc.vector.tensor_copy(acc, pt)
            else:
                nc.vector.tensor_add(
                    acc[:, :, off:], acc[:, :, off:], pt[:, :, off:])
            for h in range(HPC):
                nc.tensor.matmul(
                    ps_at[h][:, off:], v_sb[:, c, h * 128:(h + 1) * 128],
                    pt[:, h, off:],
                    start=(c == 0), stop=(c == nkc - 1))
            if c + 2 < nkc:
                pend.append(score_pair(c + 2))

        # per-head softmax denominators -> [1, 512] psum row (base
        # partition 0), copied rounded to SBUF, rank-1 broadcast across
        # partitions, reciprocal'd into SBUF; at = ps_at * (1/denom)
        # drains the attention psum (its single PSUM operand).
        at_tiles = []
        for h in range(HPC):
            ps_sum = ps.tile([128, QSB], F32, tag="psO", bufs=2,
                             name=f"pssum{h}")
            nc.tensor.matmul(ps_sum[0:1, :], ones_col, acc[:, h],
                             start=True, stop=True)
            den = work.tile([1, QSB], F32R, tag="den", bufs=1,
                            name=f"den{h}")
            nc.vector.tensor_copy(den, ps_sum[0:1, :])
            ps_bc = ps.tile([128, QSB], F32, tag="psO", bufs=2,
                            name=f"psbc{h}")
            nc.tensor.matmul(ps_bc, ones_row, den, start=True, stop=True)
            inv_bc = work.tile([128, QSB], F32, tag="invbc", bufs=2,
                               name=f"invbc{h}")
            with nc.allow_low_precision(
                    reason="~18-bit reciprocal: plenty for 1e-2 tolerance"):
                nc.vector.reciprocal_approx_fast(inv_bc, ps_bc)
            at = work.tile([128, QSB], F32R, tag="at", bufs=3,
                           name=f"at{h}")
            nc.vector.tensor_mul(at, ps_at[h], inv_bc)
            at_tiles.append(at)
        at_prev = at_tiles
    _o_proj(nc, env, b, NQSB - 1, at_prev)


def _batches(nc, env):
    big = env["big"]
    # qt/kt/v are written by batch b+1's projections only after batch b's
    # attention has fully consumed them; the PE runs batches in order, so
    # single-buffered residents are safe and save SBUF.
    env["qt_sb"] = big.tile([128, HPC, S], F32R, tag="qt", bufs=1, name="qt")
    env["kt_sb"] = big.tile([128, HPC, S], F32R, tag="kt", bufs=1, name="kt")
    env["v_sb"] = big.tile([128, S // 128, HD], F16, tag="v", bufs=1, name="v")
    for b in range(B):
        _proj(nc, env, b)
        _attn(nc, env, b)


def kernel(x, w_q, w_k, w_v, w_o, _trace=False):
    x = np.ascontiguousarray(np.asarray(x, dtype=np.float32))
    xT = np.ascontiguousarray(x.transpose(0, 2, 1))
    in_maps = []
    for c in range(N_CORES):
        sl = slice(c * HD, (c + 1) * HD)
        in_maps.append({
            "xT": xT,
            "wq": np.ascontiguousarray(np.asarray(w_q, np.float32)[:, sl]),
            "wk": np.ascontiguousarray(np.asarray(w_k, np.float32)[:, sl]),
            "wv": np.ascontiguousarray(np.asarray(w_v, np.float32)[:, sl]),
            "wo": np.ascontiguousarray(np.asarray(w_o, np.float32)[sl, :]),
        })
    if "nc" not in _CACHED:
        _CACHED["nc"] = build()
    res = run_bass_kernel_spmd(
        _CACHED["nc"], in_maps, core_ids=list(range(N_CORES)),
        trace=_trace)
    if _trace:
        _CACHED["last_result"] = res
    acc = np.zeros((B, S, D), dtype=np.float64)
    for r in res.results:
        acc += r["out"]
    return acc.astype(np.float32)


# revision 26
# speedup vs baseline: 1.2441x; 1.0318x over previous
"""Causal multi-head self-attention on 8 Trainium2 NeuronCores.

Problem: x[4, 2048, 2048] fp32, w_q/w_k/w_v/w_o [2048, 2048] fp32,
16 heads x d_head=128. out = softmax(causal(QK^T/sqrt(128))) V @ w_o.

Sharding: tensor-parallel over heads. Core c owns heads {2c, 2c+1}:
computes Q^T/K^T ([d_head, tokens]) and V ([tokens, d_head]) for its
heads from the full x (streamed pre-transposed as x^T), runs per-head
causal attention producing attnT [d_head, tokens], then the partial
output o_c = attn_c @ w_o[head rows]; the host sums the 8 partials.

v2 design notes (PE-occupancy focused — TRN2 drops the PE clock from
2.4 to 1.2 GHz after any stall, so the whole schedule is built to
keep the tensor engine continuously fed):
- Probabilities ride in fp16: exp(scale*s - 8) fits comfortably
  (scores are O(+-6)); denominators/numerators are both scaled by
  e^-8 so the ratio is unchanged. fp16 halves DVE cost (2x/4x modes)
  and is full-rate on the PE.
- Scores for BOTH heads of a chunk land in one [128, 2, 512] PSUM
  pair-tile -> ONE exp activation per chunk covers both heads, so the
  Act engine has 2x latency slack vs the PE's score+PV work.
- Causal masking multiplies by constant 0/1 fp16 mask tiles on the
  DVE (fast mode) instead of gpsimd affine_select per tile.
- Softmax denominator: fp16 chunk accumulation (DVE) then one PE
  matmul pair (selector columns) gives both heads' row sums in a
  [2, 512] PSUM tile; reciprocal_approx_fast (single DVE op, ~18
  bits) replaces the 3.3us-per-call exact reciprocal; a rank-1 PE
  matmul broadcasts 1/sum across partitions; the at = ps_at * inv
  multiply drains attention PSUM.
- O-projection of block j is deferred until after block j+1's first
  two score pairs are emitted, so the PE chews fresh score work while
  the denominator chain (DVE) finishes - no per-block PE bubble.
- PSUM->SBUF output drains alternate DVE/gpsimd so neither engine
  rate-limits the O-projection matmul stream.
- PSUM budget exactly 8 banks: psS 2x[128,2,512] (4) + psAT 2x[128,
  512] (2) + psO 2x[128,512] (2); projections reuse the psS tag.
"""

import contextlib
from collections import deque

import numpy as np

import concourse.bass as bass
from concourse import bacc
import concourse.mybir as mybir
from concourse.tile import TileContext
from concourse.bass_utils import run_bass_kernel_spmd

B = 4
S = 2048
D = 2048
NH = 16
DH = 128
N_CORES = 8
HPC = NH // N_CORES          # heads per core = 2
HD = HPC * DH                # head dims per core = 256
KO = D // 128                # contraction chunks = 16
TSB = 512                    # projection token superblock
NSB = S // TSB               # 4
QSB = 512                    # attention q superblock
NQSB = S // QSB              # 4
SCALE = float(1.0 / np.sqrt(DH))
CSHIFT = -8.0                # exp(s*scale - 8): keeps fp16 probs in range

F32 = mybir.dt.float32
F32R = mybir.dt.float32r
F16 = mybir.dt.float16

_CACHED = {}


def build(loop_r: int | None = None):
    nc = bacc.Bacc("TRN2", target_bir_lowering=False, debug=False)
    xT = nc.dram_tensor("xT", [B, D, S], F32, kind="ExternalInput")
    wq = nc.dram_tensor("wq", [D, HD], F32, kind="ExternalInput")
    wk = nc.dram_tensor("wk", [D, HD], F32, kind="ExternalInput")
    wv = nc.dram_tensor("wv", [D, HD], F32, kind="ExternalInput")
    wo = nc.dram_tensor("wo", [HD, D], F32, kind="ExternalInput")
    out = nc.dram_tensor("out", [B, S, D], F32, kind="ExternalOutput")

    env = {
        "xT_v": xT.ap().bitcast(F32R),
        "out": out,
    }

    with TileContext(nc) as tc:
        with tc.tile_pool(name="const", bufs=1) as const, \
             tc.tile_pool(name="big", bufs=1) as big, \
             tc.tile_pool(name="stream", bufs=2) as stream, \
             tc.tile_pool(name="work", bufs=4) as work, \
             tc.tile_pool(name="ps", bufs=2, space="PSUM") as ps:

            # ---- weights / constants (resident) ----
            wq_sb = const.tile([128, KO, HD], F32R)
            wk_sb = const.tile([128, KO, HD], F32R)
            wv_sb = const.tile([128, KO, HD], F32R)
            wo_sb = const.tile([128, HPC, D], F32R)

            env.update(wq_sb=wq_sb, wk_sb=wk_sb, wv_sb=wv_sb, wo_sb=wo_sb,
                       big=big, stream=stream, work=work, ps=ps)

            # Batch-0 superblock-0 xt is DMA'd interleaved with wq so the
            # first Q matmul chain starts ~2MB in, not after all weights.
            xt0 = stream.tile([128, KO, TSB], F32R, tag="xt", bufs=2)
            wq_v = wq.ap().rearrange("(ko p) m -> p ko m", p=128).bitcast(F32R)
            wk_v = wk.ap().rearrange("(ko p) m -> p ko m", p=128).bitcast(F32R)
            wv_v = wv.ap().rearrange("(ko p) m -> p ko m", p=128).bitcast(F32R)
            xT_v = env["xT_v"]
            for ko in range(KO):
                nc.sync.dma_start(wq_sb[:, ko], wq_v[:, ko])
                nc.sync.dma_start(xt0[:, ko], xT_v[0, ko * 128:(ko + 1) * 128,
                                                  0:TSB])
            for ko in range(KO):
                nc.sync.dma_start(wk_sb[:, ko], wk_v[:, ko])
            for ko in range(KO):
                nc.sync.dma_start(wv_sb[:, ko], wv_v[:, ko])
            nc.sync.dma_start(
                wo_sb, wo.ap().rearrange("(c p) n -> p c n", p=128).bitcast(F32R))
            env["xt0"] = xt0

            # ones_row for the rank-1 partition broadcast of 1/denominator
            tmp1 = const.tile([128, 1], F32)
            ones_row = const.tile([1, 128], F32R)
            nc.vector.memset(tmp1, 1.0)
            nc.vector.tensor_copy(ones_row, tmp1[0:1, 0:1].to_broadcast([1, 128]))
            # per-partition bias column for exp(s*scale + CSHIFT)
            cbias = const.tile([128, 1], F32)
            nc.vector.memset(cbias, CSHIFT)
            env["cbias"] = cbias

            # fp16 ones column for the per-head row-sum matmul
            ones_col = const.tile([128, 1], F16)
            nc.vector.tensor_copy(ones_col, tmp1)

            # causal masks (0/1), both heads' halves identical. For a
            # diagonal chunk at d = c*128 - j*512, column g of the q-block
            # is kept iff g >= d + p. One mask tile per d in {0,..,384}.
            mtmp = const.tile([128, 512], F32)
            masks = []
            for d in range(0, 512, 128):
                mk = const.tile([128, 2, 512], F16, name=f"mask{d}")
                nc.vector.memset(mtmp, 1.0)
                nc.gpsimd.affine_select(
                    out=mtmp, in_=mtmp, compare_op=mybir.AluOpType.is_ge,
                    fill=0.0, base=-d, pattern=[[1, 512]],
                    channel_multiplier=-1)
                nc.vector.tensor_copy(mk[:, 0], mtmp)
                nc.vector.tensor_copy(mk[:, 1], mtmp)
                masks.append(mk)

            env.update(ones_row=ones_row, ones_col=ones_col, masks=masks)

            loop_cm = (tc.For_i(0, loop_r, 1) if loop_r else
                       contextlib.nullcontext())
            with loop_cm:
                _batches(nc, env)

    nc.compile()
    return nc


def _proj(nc, env, b, pending_oproj=None):
    """Q^T/K^T/V projections for batch b into qt/kt/v SBUF tiles.

    pending_oproj = (prev_b, at_tiles): the previous batch's last
    O-projection, emitted after superblock 0's Q matmul chain so ~7us of
    independent PE work covers its denominator-chain latency.
    """
    stream, work, ps = env["stream"], env["work"], env["ps"]
    wq_sb, wk_sb, wv_sb = env["wq_sb"], env["wk_sb"], env["wv_sb"]
    xT_v = env["xT_v"]
    qt_sb, kt_sb, v_sb = env["qt_sb"], env["kt_sb"], env["v_sb"]

    for s in range(NSB):
        if b == 0 and s == 0:
            xt = env["xt0"]          # pre-loaded interleaved with wq
        else:
            xt = stream.tile([128, KO, TSB], F32R, tag="xt", bufs=2)
            for ko in range(KO):
                nc.sync.dma_start(
                    xt[:, ko],
                    xT_v[b, ko * 128:(ko + 1) * 128, s * TSB:(s + 1) * TSB])
        # Q^T then K^T: [dh, tokens] = W_chunk^T @ x^T; both 128-rows of
        # head-dim go in one [128, 2, 512] psum pair-tile.
        for (wsb, dst) in ((wq_sb, qt_sb), (wk_sb, kt_sb)):
            psq = ps.tile([128, 2, TSB], F32, tag="psS", bufs=2)
            for m in range(HPC):
                for ko in range(KO):
                    nc.tensor.matmul(
                        psq[:, m], wsb[:, ko, m * 128:(m + 1) * 128], xt[:, ko],
                        start=(ko == 0), stop=(ko == KO - 1))
            nc.scalar.copy(dst[:, :, s * TSB:(s + 1) * TSB], psq)
            if s == 0 and wsb is wq_sb and pending_oproj is not None:
                _o_proj(nc, env, pending_oproj[0], NQSB - 1, pending_oproj[1])
        # V: [tokens, dh] = x @ W_v, two 128-token chunks per pair-tile,
        # downcast to fp16 on drain.
        for tp in range(TSB // 256):
            psv = ps.tile([128, 2, TSB], F32, tag="psS", bufs=2)
            for ti in range(2):
                t = tp * 2 + ti
                for ko in range(KO):
                    nc.tensor.matmul(
                        psv[:, ti, 0:HD],
                        xt[:, ko, t * 128:(t + 1) * 128], wv_sb[:, ko],
                        start=(ko == 0), stop=(ko == KO - 1))
            tc0 = s * (TSB // 128) + tp * 2
            nc.scalar.copy(v_sb[:, tc0:tc0 + 2, :], psv[:, :, 0:HD])


def _o_proj(nc, env, b, j, at_tiles):
    """Fused output projection for q-superblock j of batch b."""
    work, ps = env["work"], env["ps"]
    wo_sb, out = env["wo_sb"], env["out"]
    for t in range(QSB // 128):
        for n in range(D // 512):
            ps_o = ps.tile([128, 512], F32, tag="psO", bufs=2)
            for h in range(HPC):
                nc.tensor.matmul(
                    ps_o, at_tiles[h][:, t * 128:(t + 1) * 128],
                    wo_sb[:, h, n * 512:(n + 1) * 512],
                    start=(h == 0), stop=(h == HPC - 1))
            o_st = work.tile([128, 512], F32, tag="ost", bufs=4)
            if (t * 4 + n) % 2 == 0:
                nc.vector.tensor_copy(o_st, ps_o)
            else:
                nc.scalar.copy(o_st, ps_o)
            nc.sync.dma_start(
                out.ap()[b, j * QSB + t * 128:j * QSB + (t + 1) * 128,
                         n * 512:(n + 1) * 512], o_st)


def _attn(nc, env, b):
    """Causal attention + deferred fused O-projection for batch b."""
    work, ps = env["work"], env["ps"]
    qt_sb, kt_sb, v_sb = env["qt_sb"], env["kt_sb"], env["v_sb"]
    ones_row, ones_col = env["ones_row"], env["ones_col"]
    masks = env["masks"]

    at_prev = None
    for j in range(NQSB):
        nkc = 4 * (j + 1)

        def score_pair(c):
            # causal narrowing: on diagonal chunks only q-cols >= c*128
            # matter; keep matmul width >= 256 for f32r full rate.
            if c >= 4 * j:
                off = min(c * 128 - j * QSB, QSB - 256)
            else:
                off = 0
            pss = ps.tile([128, 2, QSB], F32, tag="psS", bufs=2)
            for h in range(HPC):
                nc.tensor.matmul(
                    pss[:, h, off:], kt_sb[:, h, c * 128:(c + 1) * 128],
                    qt_sb[:, h, j * QSB + off:(j + 1) * QSB],
                    start=True, stop=True)
            return pss, off

        # four score pairs in flight before the deferred O-projection:
        # ~3.5us of PE work covering the previous block's denominator
        # chain (rowsum -> den copy -> broadcast -> recip -> at-mul).
        npre = min(4, nkc)
        pend = deque()
        for c in range(npre):
            pend.append(score_pair(c))
        if at_prev is not None:
            _o_proj(nc, env, b, j - 1, at_prev)

        acc = work.tile([128, 2, QSB], F16, tag="acc", bufs=2)
        ps_at = [ps.tile([128, QSB], F32, tag="psAT", bufs=2, name=f"psat{h}")
                 for h in range(HPC)]
        for c in range(nkc):
            pss, off = pend.popleft()
            pt = work.tile([128, 2, QSB], F16, tag="pt", bufs=3)
            nc.scalar.activation(
                pt[:, :, off:], pss[:, :, off:],
                mybir.ActivationFunctionType.Exp, bias=env["cbias"],
                scale=SCALE)
            if c >= 4 * j:
                # causal: multiply by the 0/1 mask for this diagonal chunk
                msk = masks[(c * 128 - j * QSB) // 128]
                nc.vector.tensor_mul(
                    pt[:, :, off:], pt[:, :, off:], msk[:, :, off:])
            if c == 0:
                nc.vector.tensor_copy(acc, pt)
            else:
                nc.vector.tensor_add(
                    acc[:, :, off:], acc[:, :, off:], pt[:, :, off:])
            for h in range(HPC):
                nc.tensor.matmul(
                    ps_at[h][:, off:], v_sb[:, c, h * 128:(h + 1) * 128],
                    pt[:, h, off:],
                    start=(c == 0), stop=(c == nkc - 1))
            if c + npre < nkc:
                pend.append(score_pair(c + npre))

        # per-head softmax denominators -> [1, 512] psum row (base
        # partition 0), copied rounded to SBUF, rank-1 broadcast across
        # partitions, reciprocal'd into SBUF; at = ps_at * (1/denom)
        # drains the attention psum (its single PSUM operand).
        at_tiles = []
        for h in range(HPC):
            ps_sum = ps.tile([128, QSB], F32, tag="psO", bufs=2,
                             name=f"pssum{h}")
            nc.tensor.matmul(ps_sum[0:1, :], ones_col, acc[:, h],
                             start=True, stop=True)
            den = work.tile([1, QSB], F32R, tag="den", bufs=1,
                            name=f"den{h}")
            nc.scalar.copy(den, ps_sum[0:1, :])
            ps_bc = ps.tile([128, QSB], F32, tag="psO", bufs=2,
                            name=f"psbc{h}")
            nc.tensor.matmul(ps_bc, ones_row, den, start=True, stop=True)
            inv_bc = work.tile([128, QSB], F32, tag="invbc", bufs=2,
                               name=f"invbc{h}")
            with nc.allow_low_precision(
                    reason="~18-bit reciprocal: plenty for 1e-2 tolerance"):
                nc.vector.reciprocal_approx_fast(inv_bc, ps_bc)
            at = work.tile([128, QSB], F32R, tag="at", bufs=2,
                           name=f"at{h}")
            nc.vector.tensor_mul(at, ps_at[h], inv_bc)
            at_tiles.append(at)
        at_prev = at_tiles
    return at_prev


def _batches(nc, env):
    big = env["big"]
    # qt/kt/v are written by batch b+1's projections only after batch b's
    # attention has fully consumed them; the PE runs batches in order, so
    # single-buffered residents are safe and save SBUF.
    env["qt_sb"] = big.tile([128, HPC, S], F32R, tag="qt", bufs=1, name="qt")
    env["kt_sb"] = big.tile([128, HPC, S], F32R, tag="kt", bufs=1, name="kt")
    env["v_sb"] = big.tile([128, S // 128, HD], F16, tag="v", bufs=1, name="v")
    pending = None
    for b in range(B):
        _proj(nc, env, b, pending_oproj=pending)
        at_last = _attn(nc, env, b)
        pending = (b, at_last)
    _o_proj(nc, env, B - 1, NQSB - 1, at_last)


def kernel(x, w_q, w_k, w_v, w_o, _trace=False):
    x = np.ascontiguousarray(np.asarray(x, dtype=np.float32))
    xT = np.ascontiguousarray(x.transpose(0, 2, 1))
    in_maps = []
    for c in range(N_CORES):
        sl = slice(c * HD, (c + 1) * HD)
        in_maps.append({
            "xT": xT,
            "wq": np.ascontiguousarray(np.asarray(w_q, np.float32)[:, sl]),
            "wk": np.ascontiguousarray(np.asarray(w_k, np.float32)[:, sl]),
            "wv": np.ascontiguousarray(np.asarray(w_v, np.float32)[:, sl]),
            "wo": np.ascontiguousarray(np.asarray(w_o, np.float32)[sl, :]),
        })
    if "nc" not in _CACHED:
        _CACHED["nc"] = build()
    res = run_bass_kernel_spmd(
        _CACHED["nc"], in_maps, core_ids=list(range(N_CORES)),
        trace=_trace)
    if _trace:
        _CACHED["last_result"] = res
    acc = np.zeros((B, S, D), dtype=np.float64)
    for r in res.results:
        acc += r["out"]
    return acc.astype(np.float32)


# revision 30
# speedup vs baseline: 1.3318x; 1.0705x over previous
"""Causal multi-head self-attention on 8 Trainium2 NeuronCores.

Problem: x[4, 2048, 2048] fp32, w_q/w_k/w_v/w_o [2048, 2048] fp32,
16 heads x d_head=128. out = softmax(causal(QK^T/sqrt(128))) V @ w_o.

Sharding: tensor-parallel over heads. Core c owns heads {2c, 2c+1}:
computes Q^T/K^T ([d_head, tokens]) and V ([tokens, d_head]) for its
heads from the full x (streamed pre-transposed as x^T), runs per-head
causal attention producing attnT [d_head, tokens], then the partial
output o_c = attn_c @ w_o[head rows]; the host sums the 8 partials.

v2 design notes (PE-occupancy focused — TRN2 drops the PE clock from
2.4 to 1.2 GHz after any stall, so the whole schedule is built to
keep the tensor engine continuously fed):
- Probabilities ride in fp16: exp(scale*s - 8) fits comfortably
  (scores are O(+-6)); denominators/numerators are both scaled by
  e^-8 so the ratio is unchanged. fp16 halves DVE cost (2x/4x modes)
  and is full-rate on the PE.
- Scores for BOTH heads of a chunk land in one [128, 2, 512] PSUM
  pair-tile -> ONE exp activation per chunk covers both heads, so the
  Act engine has 2x latency slack vs the PE's score+PV work.
- Causal masking multiplies by constant 0/1 fp16 mask tiles on the
  DVE (fast mode) instead of gpsimd affine_select per tile.
- Softmax denominator: fp16 chunk accumulation (DVE) then one PE
  matmul pair (selector columns) gives both heads' row sums in a
  [2, 512] PSUM tile; reciprocal_approx_fast (single DVE op, ~18
  bits) replaces the 3.3us-per-call exact reciprocal; a rank-1 PE
  matmul broadcasts 1/sum across partitions; the at = ps_at * inv
  multiply drains attention PSUM.
- O-projection of block j is deferred until after block j+1's first
  two score pairs are emitted, so the PE chews fresh score work while
  the denominator chain (DVE) finishes - no per-block PE bubble.
- PSUM->SBUF output drains alternate DVE/gpsimd so neither engine
  rate-limits the O-projection matmul stream.
- PSUM budget exactly 8 banks: psS 2x[128,2,512] (4) + psAT 2x[128,
  512] (2) + psO 2x[128,512] (2); projections reuse the psS tag.
"""

import contextlib
from collections import deque

import numpy as np

import concourse.bass as bass
from concourse import bacc
import concourse.mybir as mybir
from concourse.tile import TileContext
from concourse.bass_utils import run_bass_kernel_spmd

B = 4
S = 2048
D = 2048
NH = 16
DH = 128
N_CORES = 8
HPC = NH // N_CORES          # heads per core = 2
HD = HPC * DH                # head dims per core = 256
KO = D // 128                # contraction chunks = 16
TSB = 512                    # projection token superblock
NSB = S // TSB               # 4
QSB = 512                    # attention q superblock
NQSB = S // QSB              # 4
SCALE = float(1.0 / np.sqrt(DH))
CSHIFT = -8.0                # exp(s*scale - 8): keeps fp16 probs in range

F32 = mybir.dt.float32
F32R = mybir.dt.float32r
F16 = mybir.dt.float16

_CACHED = {}


def build(loop_r: int | None = None):
    nc = bacc.Bacc("TRN2", target_bir_lowering=False, debug=False)
    xT = nc.dram_tensor("xT", [B, D, S], F32, kind="ExternalInput")
    wq = nc.dram_tensor("wq", [D, HD], F32, kind="ExternalInput")
    wk = nc.dram_tensor("wk", [D, HD], F32, kind="ExternalInput")
    wv = nc.dram_tensor("wv", [D, HD], F32, kind="ExternalInput")
    wo = nc.dram_tensor("wo", [HD, D], F32, kind="ExternalInput")
    out = nc.dram_tensor("out", [B, S, D], F32, kind="ExternalOutput")

    env = {
        "xT_v": xT.ap().bitcast(F32R),
        "out": out,
    }

    with TileContext(nc) as tc:
        with tc.tile_pool(name="const", bufs=1) as const, \
             tc.tile_pool(name="big", bufs=1) as big, \
             tc.tile_pool(name="stream", bufs=2) as stream, \
             tc.tile_pool(name="work", bufs=4) as work, \
             tc.tile_pool(name="ps", bufs=2, space="PSUM") as ps:

            # ---- weights / constants (resident) ----
            wq_sb = const.tile([128, KO, HD], F32R)
            wk_sb = const.tile([128, KO, HD], F32R)
            wv_sb = const.tile([128, KO, HD], F32R)
            wo_sb = const.tile([128, HPC, D], F32R)

            env.update(wq_sb=wq_sb, wk_sb=wk_sb, wv_sb=wv_sb, wo_sb=wo_sb,
                       big=big, stream=stream, work=work, ps=ps)

            # Batch-0 superblock-0 xt is DMA'd interleaved with wq so the
            # first Q matmul chain starts ~2MB in, not after all weights.
            xt0 = stream.tile([128, KO, TSB], F32R, tag="xt", bufs=2)
            wq_v = wq.ap().rearrange("(ko p) m -> p ko m", p=128).bitcast(F32R)
            wk_v = wk.ap().rearrange("(ko p) m -> p ko m", p=128).bitcast(F32R)
            wv_v = wv.ap().rearrange("(ko p) m -> p ko m", p=128).bitcast(F32R)
            xT_v = env["xT_v"]
            for ko in range(KO):
                nc.sync.dma_start(wq_sb[:, ko], wq_v[:, ko])
                nc.sync.dma_start(xt0[:, ko], xT_v[0, ko * 128:(ko + 1) * 128,
                                                  0:TSB])
            for ko in range(KO):
                nc.sync.dma_start(wk_sb[:, ko], wk_v[:, ko])
            for ko in range(KO):
                nc.sync.dma_start(wv_sb[:, ko], wv_v[:, ko])
            nc.sync.dma_start(
                wo_sb, wo.ap().rearrange("(c p) n -> p c n", p=128).bitcast(F32R))
            env["xt0"] = xt0

            # per-partition bias column for exp(s*scale + CSHIFT)
            cbias = const.tile([128, 1], F32)
            nc.vector.memset(cbias, CSHIFT)
            env["cbias"] = cbias

            # all-ones [128,128] fp16: ones_mat^T @ acc_h gives the softmax
            # denominator already broadcast across every output partition
            tmpm = const.tile([128, 128], F32)
            ones_mat = const.tile([128, 128], F16)
            nc.vector.memset(tmpm, 1.0)
            nc.vector.tensor_copy(ones_mat, tmpm)

            # causal masks (0/1), both heads' halves identical. For a
            # diagonal chunk at d = c*128 - j*512, column g of the q-block
            # is kept iff g >= d + p. One mask tile per d in {0,..,384}.
            mtmp = const.tile([128, 512], F32)
            masks = []
            for d in range(0, 512, 128):
                mk = const.tile([128, 2, 512], F16, name=f"mask{d}")
                nc.vector.memset(mtmp, 1.0)
                nc.gpsimd.affine_select(
                    out=mtmp, in_=mtmp, compare_op=mybir.AluOpType.is_ge,
                    fill=0.0, base=-d, pattern=[[1, 512]],
                    channel_multiplier=-1)
                nc.vector.tensor_copy(mk[:, 0], mtmp)
                nc.vector.tensor_copy(mk[:, 1], mtmp)
                masks.append(mk)

            env.update(ones_mat=ones_mat, masks=masks)

            loop_cm = (tc.For_i(0, loop_r, 1) if loop_r else
                       contextlib.nullcontext())
            with loop_cm:
                _batches(nc, env)

    nc.compile()
    return nc


def _proj(nc, env, b, pending_oproj=None):
    """Q^T/K^T/V projections for batch b into qt/kt/v SBUF tiles.

    pending_oproj = (prev_b, at_tiles): the previous batch's last
    O-projection, emitted after superblock 0's Q matmul chain so ~7us of
    independent PE work covers its denominator-chain latency.
    """
    stream, work, ps = env["stream"], env["work"], env["ps"]
    wq_sb, wk_sb, wv_sb = env["wq_sb"], env["wk_sb"], env["wv_sb"]
    xT_v = env["xT_v"]
    qt_sb, kt_sb, v_sb = env["qt_sb"], env["kt_sb"], env["v_sb"]

    for s in range(NSB):
        if b == 0 and s == 0:
            xt = env["xt0"]          # pre-loaded interleaved with wq
        else:
            xt = stream.tile([128, KO, TSB], F32R, tag="xt", bufs=2)
            for ko in range(KO):
                nc.sync.dma_start(
                    xt[:, ko],
                    xT_v[b, ko * 128:(ko + 1) * 128, s * TSB:(s + 1) * TSB])
        # Q^T then K^T: [dh, tokens] = W_chunk^T @ x^T; both 128-rows of
        # head-dim go in one [128, 2, 512] psum pair-tile.
        for (wsb, dst) in ((wq_sb, qt_sb), (wk_sb, kt_sb)):
            psq = ps.tile([128, 2, TSB], F32, tag="psS", bufs=2)
            for m in range(HPC):
                for ko in range(KO):
                    nc.tensor.matmul(
                        psq[:, m], wsb[:, ko, m * 128:(m + 1) * 128], xt[:, ko],
                        start=(ko == 0), stop=(ko == KO - 1))
            nc.scalar.copy(dst[:, :, s * TSB:(s + 1) * TSB], psq)
            if s == 0 and wsb is wq_sb and pending_oproj is not None:
                _o_proj(nc, env, pending_oproj[0], NQSB - 1, pending_oproj[1])
        # V: [tokens, dh] = x @ W_v, two 128-token chunks per pair-tile,
        # downcast to fp16 on drain.
        for tp in range(TSB // 256):
            psv = ps.tile([128, 2, TSB], F32, tag="psS", bufs=2)
            for ti in range(2):
                t = tp * 2 + ti
                for ko in range(KO):
                    nc.tensor.matmul(
                        psv[:, ti, 0:HD],
                        xt[:, ko, t * 128:(t + 1) * 128], wv_sb[:, ko],
                        start=(ko == 0), stop=(ko == KO - 1))
            tc0 = s * (TSB // 128) + tp * 2
            nc.scalar.copy(v_sb[:, tc0:tc0 + 2, :], psv[:, :, 0:HD])


def _o_proj(nc, env, b, j, at_tiles):
    """Fused output projection for q-superblock j of batch b."""
    work, ps = env["work"], env["ps"]
    wo_sb, out = env["wo_sb"], env["out"]
    for t in range(QSB // 128):
        for n in range(D // 512):
            ps_o = ps.tile([128, 512], F32, tag="psO", bufs=2)
            for h in range(HPC):
                nc.tensor.matmul(
                    ps_o, at_tiles[h][:, t * 128:(t + 1) * 128],
                    wo_sb[:, h, n * 512:(n + 1) * 512],
                    start=(h == 0), stop=(h == HPC - 1))
            o_st = work.tile([128, 512], F32, tag="ost", bufs=4)
            if (t * 4 + n) % 2 == 0:
                nc.vector.tensor_copy(o_st, ps_o)
            else:
                nc.scalar.copy(o_st, ps_o)
            nc.sync.dma_start(
                out.ap()[b, j * QSB + t * 128:j * QSB + (t + 1) * 128,
                         n * 512:(n + 1) * 512], o_st)


def _attn(nc, env, b):
    """Causal attention + deferred fused O-projection for batch b."""
    work, ps = env["work"], env["ps"]
    qt_sb, kt_sb, v_sb = env["qt_sb"], env["kt_sb"], env["v_sb"]
    ones_mat = env["ones_mat"]
    masks = env["masks"]

    at_prev = None
    for j in range(NQSB):
        nkc = 4 * (j + 1)

        def score_pair(c):
            # causal narrowing: on diagonal chunks only q-cols >= c*128
            # matter; keep matmul width >= 256 for f32r full rate.
            if c >= 4 * j:
                off = min(c * 128 - j * QSB, QSB - 256)
            else:
                off = 0
            pss = ps.tile([128, 2, QSB], F32, tag="psS", bufs=2)
            for h in range(HPC):
                nc.tensor.matmul(
                    pss[:, h, off:], kt_sb[:, h, c * 128:(c + 1) * 128],
                    qt_sb[:, h, j * QSB + off:(j + 1) * QSB],
                    start=True, stop=True)
            return pss, off

        # four score pairs in flight before the deferred O-projection:
        # ~3.5us of PE work covering the previous block's denominator
        # chain (rowsum -> den copy -> broadcast -> recip -> at-mul).
        npre = min(4, nkc)
        pend = deque()
        for c in range(npre):
            pend.append(score_pair(c))
        if at_prev is not None:
            _o_proj(nc, env, b, j - 1, at_prev)

        acc = work.tile([128, 2, QSB], F16, tag="acc", bufs=2)
        ps_at = [ps.tile([128, QSB], F32, tag="psAT", bufs=2, name=f"psat{h}")
                 for h in range(HPC)]
        for c in range(nkc):
            pss, off = pend.popleft()
            pt = work.tile([128, 2, QSB], F16, tag="pt", bufs=3)
            nc.scalar.activation(
                pt[:, :, off:], pss[:, :, off:],
                mybir.ActivationFunctionType.Exp, bias=env["cbias"],
                scale=SCALE)
            if c >= 4 * j:
                # causal: multiply by the 0/1 mask for this diagonal chunk
                msk = masks[(c * 128 - j * QSB) // 128]
                nc.vector.tensor_mul(
                    pt[:, :, off:], pt[:, :, off:], msk[:, :, off:])
            if c == 0:
                nc.vector.tensor_copy(acc, pt)
            else:
                nc.vector.tensor_add(
                    acc[:, :, off:], acc[:, :, off:], pt[:, :, off:])
            for h in range(HPC):
                nc.tensor.matmul(
                    ps_at[h][:, off:], v_sb[:, c, h * 128:(h + 1) * 128],
                    pt[:, h, off:],
                    start=(c == 0), stop=(c == nkc - 1))
            if c + npre < nkc:
                pend.append(score_pair(c + npre))

        # per-head softmax denominator, broadcast across partitions by a
        # single ones-matrix matmul; reciprocal'd into SBUF; the
        # at = ps_at * (1/denom) multiply drains the attention psum.
        at_tiles = []
        for h in range(HPC):
            ps_bc = ps.tile([128, QSB], F32, tag="psO", bufs=2,
                            name=f"psbc{h}")
            nc.tensor.matmul(ps_bc, ones_mat, acc[:, h],
                             start=True, stop=True)
            inv_bc = work.tile([128, QSB], F32, tag="invbc", bufs=2,
                               name=f"invbc{h}")
            with nc.allow_low_precision(
                    reason="~18-bit reciprocal: plenty for 1e-2 tolerance"):
                nc.vector.reciprocal_approx_fast(inv_bc, ps_bc)
            at = work.tile([128, QSB], F32R, tag="at", bufs=2,
                           name=f"at{h}")
            nc.vector.tensor_mul(at, ps_at[h], inv_bc)
            at_tiles.append(at)
        at_prev = at_tiles
    return at_prev


def _batches(nc, env):
    big = env["big"]
    # qt/kt/v are written by batch b+1's projections only after batch b's
    # attention has fully consumed them; the PE runs batches in order, so
    # single-buffered residents are safe and save SBUF.
    env["qt_sb"] = big.tile([128, HPC, S], F32R, tag="qt", bufs=1, name="qt")
    env["kt_sb"] = big.tile([128, HPC, S], F32R, tag="kt", bufs=1, name="kt")
    env["v_sb"] = big.tile([128, S // 128, HD], F16, tag="v", bufs=1, name="v")
    pending = None
    for b in range(B):
        _proj(nc, env, b, pending_oproj=pending)
        at_last = _attn(nc, env, b)
        pending = (b, at_last)
    _o_proj(nc, env, B - 1, NQSB - 1, at_last)


def kernel(x, w_q, w_k, w_v, w_o, _trace=False):
    x = np.ascontiguousarray(np.asarray(x, dtype=np.float32))
    xT = np.ascontiguousarray(x.transpose(0, 2, 1))
    in_maps = []
    for c in range(N_CORES):
        sl = slice(c * HD, (c + 1) * HD)
        in_maps.append({
            "xT": xT,
            "wq": np.ascontiguousarray(np.asarray(w_q, np.float32)[:, sl]),
            "wk": np.ascontiguousarray(np.asarray(w_k, np.float32)[:, sl]),
            "wv": np.ascontiguousarray(np.asarray(w_v, np.float32)[:, sl]),
            "wo": np.ascontiguousarray(np.asarray(w_o, np.float32)[sl, :]),
        })
    if "nc" not in _CACHED:
        _CACHED["nc"] = build()
    res = run_bass_kernel_spmd(
        _CACHED["nc"], in_maps, core_ids=list(range(N_CORES)),
        trace=_trace)
    if _trace:
        _CACHED["last_result"] = res
    acc = np.zeros((B, S, D), dtype=np.float64)
    for r in res.results:
        acc += r["out"]
    return acc.astype(np.float32)
